# revision 1
# baseline (speedup 1.0000x reference)
"""Trainium2 Bass kernel for nn_Encoder_31550829756513 (2-layer dual-branch GCN).

Strategy (8 NeuronCores, node-partitioned graph parallel):
  - Host: build sym-norms for both branches, append self loops, sort edges by
    destination block (ascending source within block), pack per-(core, slot)
    128-edge chunk streams with a shared compile-time chunk schedule K[slot].
  - Device, per core (single SPMD program, all offsets static):
      hw = x_local @ W1                       (PE, fp16 in / fp32 acc)
      AllGather hw -> full fp16 table [NPAD, H]
      L1: per chunk: 128-row indirect DMA gather (one row per partition,
          offset column [P,1]), selector = (iota == dstloc) on DVE, scale
          rows by [gnorm|pnorm] in one [P,2,H] op, one matmul accumulates
          [z_g|z_p] in PSUM; relu(z+b1) -> h1 table (fp16, g|p interleaved)
      AllGather h1 -> full fp16 table [NPAD, 2H]
      L2: same message pass on h1; per block: logits via (W2@dense_w) dot,
          softmax-of-2 == sigmoid(lg-lp), blend in h-space, PE transpose,
          @W2 + b2
  - Host: concatenate output shards, slice to N rows.
"""

import os
import numpy as np

P = 128

_FP16 = np.float16


class Cfg:
    def __init__(self, n, e, d=256, h=128, o=64, ncores=8, gb=7, gg=5):
        self.N = n
        self.E = e
        self.D = d
        self.H = h
        self.O = o
        self.ncores = ncores
        self.NBLK = -(-n // P)
        self.NB = -(-self.NBLK // ncores)
        self.CORE_ROWS = self.NB * P
        self.NPAD = ncores * self.CORE_ROWS
        self.GB = gb
        self.GG = gg


FULL = Cfg(100000, 1600000)


# ----------------------------------------------------------------------------
# Host preprocessing
# ----------------------------------------------------------------------------

def _preprocess(cfg, x, edge_index, ppmi_edge_weight, W1, b1, W2, b2,
                dense_w, dense_b):
    n, e = cfg.N, cfg.E
    row = np.asarray(edge_index[0], dtype=np.int64).astype(np.int32)
    col = np.asarray(edge_index[1], dtype=np.int64).astype(np.int32)
    ppmi = np.asarray(ppmi_edge_weight, dtype=np.float64)

    sl = np.arange(n, dtype=np.int32)
    row_sl = np.concatenate([row, sl])
    col_sl = np.concatenate([col, sl])
    ones_n = np.ones(n, dtype=np.float64)

    def sym_dis(ew):
        deg = np.bincount(row_sl, weights=ew, minlength=n)
        return np.where(deg > 0, deg ** -0.5, 0.0)

    dis_g = sym_dis(np.concatenate([np.ones(e), ones_n]))
    dis_p = sym_dis(np.concatenate([ppmi, ones_n]))
    # real (non-self-loop) edges only; self loops handled per-block on device
    gnorm = (dis_g[row] * dis_g[col]).astype(np.float32)
    pnorm = (dis_p[row] * ppmi * dis_p[col]).astype(np.float32)

    # per-node self-loop norms (dis^2), per core layout [P, NB, 2]
    selfn = np.zeros((cfg.NPAD, 2), dtype=np.float32)
    selfn[:n, 0] = (dis_g * dis_g).astype(np.float32)
    selfn[:n, 1] = (dis_p * dis_p).astype(np.float32)
    self_stream = np.ascontiguousarray(
        selfn.reshape(cfg.ncores, cfg.NB, P, 2).transpose(0, 2, 1, 3))

    blk_all = col >> 7
    order = np.lexsort((row, blk_all))   # by dst block, ascending src
    src_s = row[order]
    dst_s = col[order]
    gn_s = gnorm[order]
    pn_s = pnorm[order]
    blk = blk_all[order]
    dstloc = (dst_s & 127).astype(np.float32)

    bcnt = np.bincount(blk, minlength=cfg.NBLK)
    bstart = np.concatenate([[0], np.cumsum(bcnt)[:-1]])
    rank = np.arange(src_s.shape[0], dtype=np.int64) - bstart[blk]

    core_of = blk // cfg.NB
    slot_of = blk - core_of * cfg.NB

    cnt = np.zeros((cfg.ncores, cfg.NB), dtype=np.int64)
    bids = np.arange(cfg.NBLK)
    cnt[bids // cfg.NB, bids % cfg.NB] = bcnt
    K = np.maximum(1, -(-cnt.max(axis=0) // P))
    C0 = np.concatenate([[0], np.cumsum(K)[:-1]])
    NCH = int(K.sum())

    idx_stream = np.zeros((cfg.ncores, P, NCH), dtype=np.int32)
    dst_stream = np.zeros((cfg.ncores, P, NCH), dtype=np.float32)
    nrm_stream = np.zeros((cfg.ncores, P, NCH, 2), dtype=np.float32)

    k_e = rank >> 7
    p_e = (rank & 127).astype(np.int64)
    c_arr = core_of.astype(np.int64)
    chcol = C0[slot_of] + k_e
    idx_stream[c_arr, p_e, chcol] = src_s
    dst_stream[c_arr, p_e, chcol] = dstloc
    nrm_stream[c_arr, p_e, chcol, 0] = gn_s
    nrm_stream[c_arr, p_e, chcol, 1] = pn_s

    xT = np.zeros((cfg.D, cfg.NPAD), dtype=_FP16)
    xT[:, :n] = np.asarray(x, dtype=np.float32).T.astype(_FP16)

    W1f = np.asarray(W1, dtype=np.float32).astype(_FP16)
    W2f = np.asarray(W2, dtype=np.float32).astype(_FP16)
    b1r = np.tile(np.asarray(b1, dtype=np.float32)[None, :], (P, 1))
    b2r = np.tile(np.asarray(b2, dtype=np.float32)[None, :], (P, 1))
    w2d = (np.asarray(W2, np.float64) @ np.asarray(dense_w, np.float64)).ravel()
    w2dr = np.tile(w2d.astype(np.float32)[None, :], (P, 1))

    in_maps = []
    for c in range(cfg.ncores):
        in_maps.append({
            "xT": np.ascontiguousarray(
                xT[:, c * cfg.CORE_ROWS:(c + 1) * cfg.CORE_ROWS]),
            "w1": W1f, "w2": W2f, "b1r": b1r, "b2r": b2r, "w2dr": w2dr,
            "idxs": idx_stream[c], "dsts": dst_stream[c],
            "nrms": nrm_stream[c], "selfn": self_stream[c],
        })
    return in_maps, tuple(int(k) for k in K)


# ----------------------------------------------------------------------------
# Device program
# ----------------------------------------------------------------------------

def build_program(cfg, K):
    from concourse import bass, mybir, tile, bacc
    from concourse.masks import make_identity

    dt16 = mybir.dt.float16
    dt32 = mybir.dt.float32
    AOT = mybir.AluOpType

    NCH = int(sum(K))
    C0 = [0]
    for k in K[:-1]:
        C0.append(C0[-1] + k)
    NB, H, O, D = cfg.NB, cfg.H, cfg.O, cfg.D

    nc = bacc.Bacc("TRN2", debug=False, enable_asserts=False,
                   num_devices=cfg.ncores)

    xT = nc.dram_tensor("xT", [D, cfg.CORE_ROWS], dt16, kind="ExternalInput")
    w1 = nc.dram_tensor("w1", [D, H], dt16, kind="ExternalInput")
    w2 = nc.dram_tensor("w2", [H, O], dt16, kind="ExternalInput")
    b1r = nc.dram_tensor("b1r", [P, H], dt32, kind="ExternalInput")
    b2r = nc.dram_tensor("b2r", [P, O], dt32, kind="ExternalInput")
    w2dr = nc.dram_tensor("w2dr", [P, H], dt32, kind="ExternalInput")
    idxs = nc.dram_tensor("idxs", [P, NCH], mybir.dt.int32,
                          kind="ExternalInput")
    dsts = nc.dram_tensor("dsts", [P, NCH], dt32, kind="ExternalInput")
    nrms = nc.dram_tensor("nrms", [P, NCH, 2], dt32, kind="ExternalInput")
    selfn = nc.dram_tensor("selfn", [P, NB, 2], dt32, kind="ExternalInput")
    outp = nc.dram_tensor("out", [cfg.CORE_ROWS, O], dt32,
                          kind="ExternalOutput")

    hw_shard = nc.dram_tensor("hw_shard", [cfg.CORE_ROWS, H], dt16)
    hw_full = nc.dram_tensor("hw_full", [cfg.NPAD, H], dt16,
                             addr_space="Shared")
    h1_shard = nc.dram_tensor("h1_shard", [cfg.CORE_ROWS, 2 * H], dt16)
    h1_full = nc.dram_tensor("h1_full", [cfg.NPAD, 2 * H], dt16,
                             addr_space="Shared")

    groups_all = [list(range(cfg.ncores))]
    groups = [(g0, min(cfg.GG, NB - g0)) for g0 in range(0, NB, cfg.GG)]

    with tile.TileContext(nc) as tc:
        with tc.tile_pool(name="const", bufs=1) as cpool:
            w1a = cpool.tile([P, H], dt16)
            w1b = cpool.tile([P, H], dt16)
            nc.sync.dma_start(out=w1a[:], in_=w1[0:P, :])
            nc.sync.dma_start(out=w1b[:], in_=w1[P:2 * P, :])
            w2sb = cpool.tile([P, O], dt16)
            nc.sync.dma_start(out=w2sb[:], in_=w2[:, :])
            b1sb = cpool.tile([P, H], dt32)
            nc.sync.dma_start(out=b1sb[:], in_=b1r[:, :])
            b2sb = cpool.tile([P, O], dt32)
            nc.sync.dma_start(out=b2sb[:], in_=b2r[:, :])
            w2dsb = cpool.tile([P, H], dt32)
            nc.sync.dma_start(out=w2dsb[:], in_=w2dr[:, :])
            it16 = cpool.tile([P, P], mybir.dt.int16)
            nc.gpsimd.iota(it16[:], pattern=[[1, P]], base=0,
                           channel_multiplier=0)
            iotaf = cpool.tile([P, P], dt16)
            nc.vector.tensor_copy(out=iotaf[:], in_=it16[:])
            ident = cpool.tile([P, P], dt32)
            make_identity(nc, ident[:])
            ident16 = cpool.tile([P, P], dt16)
            make_identity(nc, ident16[:])
            self_sb = cpool.tile([P, NB, 2], dt32)
            nc.sync.dma_start(out=self_sb[:], in_=selfn[:, :, :])
            idx_sb = cpool.tile([P, NCH], mybir.dt.int32)
            nc.sync.dma_start(out=idx_sb[:], in_=idxs[:, :])
            dst_sb = cpool.tile([P, NCH], dt32)
            nc.sync.dma_start(out=dst_sb[:], in_=dsts[:, :])
            nrm_sb = cpool.tile([P, NCH, 2], dt32)
            nc.sync.dma_start(out=nrm_sb[:], in_=nrms[:, :, :])

            # ---------------- phase B: hw = x @ W1 ----------------
            hw_view = hw_shard.ap().rearrange("(t p) h -> p t h", p=P)
            with tc.tile_pool(name="phB", bufs=3) as xp, \
                 tc.tile_pool(name="phBp", bufs=4, space="PSUM") as bp, \
                 tc.tile_pool(name="phBh", bufs=2) as hp:
                for g0 in range(0, NB, cfg.GB):
                    gs = min(cfg.GB, NB - g0)
                    xa = xp.tile([P, gs * P], dt16, tag="xa")
                    xb = xp.tile([P, gs * P], dt16, tag="xb")
                    nc.sync.dma_start(out=xa[:],
                                      in_=xT[0:P, g0 * P:(g0 + gs) * P])
                    nc.sync.dma_start(out=xb[:],
                                      in_=xT[P:2 * P, g0 * P:(g0 + gs) * P])
                    hwg = hp.tile([P, gs, H], dt16, tag="hwg")
                    for t in range(gs):
                        ps = bp.tile([P, H], dt32, tag="bps")
                        nc.tensor.matmul(out=ps[:],
                                         lhsT=xa[:, t * P:(t + 1) * P],
                                         rhs=w1a[:], start=True, stop=False)
                        nc.tensor.matmul(out=ps[:],
                                         lhsT=xb[:, t * P:(t + 1) * P],
                                         rhs=w1b[:], start=False, stop=True)
                        nc.vector.tensor_copy(out=hwg[:, t, :], in_=ps[:])
                    nc.sync.dma_start(out=hw_view[:, g0:g0 + gs, :], in_=hwg[:])

            nc.gpsimd.collective_compute(
                "AllGather", AOT.bypass, replica_groups=groups_all,
                ins=[hw_shard.ap().opt()], outs=[hw_full.ap().opt()])

            # ---------------- message-pass helper ----------------
            def msg_pass(table, shard_view, width2, out_cb, flush_cb, gpool,
                         zpool, spool, rpool, opool, otag, oshape_inner,
                         odtype):
                nbr = 2 if width2 else 1
                for (g0, gs) in groups:
                    c0 = C0[g0]
                    kg = sum(K[g0 + sl] for sl in range(gs))
                    gth = gpool.tile([P, kg, nbr, H], dt16, tag="gth")
                    for cr in range(kg):
                        nc.gpsimd.indirect_dma_start(
                            out=gth[:, cr, :, :].rearrange("p b h -> p (b h)"),
                            out_offset=None,
                            in_=table.ap(),
                            in_offset=bass.IndirectOffsetOnAxis(
                                ap=idx_sb[:, c0 + cr:c0 + cr + 1], axis=0))
                    selfb = gpool.tile([P, gs, nbr, H], dt16, tag="selfb")
                    nc.sync.dma_start(
                        out=selfb[:].rearrange("p t b h -> p t (b h)"),
                        in_=shard_view[:, g0:g0 + gs, :])
                    og = opool.tile([P, gs] + oshape_inner, odtype, tag=otag)
                    for sl in range(gs):
                        s = g0 + sl
                        z = zpool.tile([P, 2 * H], dt32, tag="z")
                        for k in range(K[s]):
                            c = C0[s] + k
                            cr = c - c0
                            sel = spool.tile([P, P], dt16, tag="sel")
                            nc.vector.tensor_scalar(
                                out=sel[:], in0=iotaf[:],
                                scalar1=dst_sb[:, c:c + 1],
                                scalar2=None, op0=AOT.is_equal)
                            rw = rpool.tile([P, 2, H], dt16, tag="rw")
                            gsrc = gth[:, cr, 1 if width2 else 0, :]
                            nc.vector.tensor_scalar(
                                out=rw[:, 0, :], in0=gth[:, cr, 0, :],
                                scalar1=nrm_sb[:, c, 0:1],
                                scalar2=None, op0=AOT.mult)
                            nc.vector.tensor_scalar(
                                out=rw[:, 1, :], in0=gsrc,
                                scalar1=nrm_sb[:, c, 1:2],
                                scalar2=None, op0=AOT.mult)
                            nc.tensor.matmul(
                                out=z[:], lhsT=sel[:],
                                rhs=rw[:].rearrange("p b h -> p (b h)"),
                                start=(k == 0), stop=False)
                        # self-loop contribution: diagonal (identity) matmul
                        # over this core's own contiguous shard rows
                        rwS = rpool.tile([P, 2, H], dt16, tag="rw")
                        ssrc = selfb[:, sl, 1 if width2 else 0, :]
                        nc.vector.tensor_scalar(
                            out=rwS[:, 0, :], in0=selfb[:, sl, 0, :],
                            scalar1=self_sb[:, s, 0:1],
                            scalar2=None, op0=AOT.mult)
                        nc.vector.tensor_scalar(
                            out=rwS[:, 1, :], in0=ssrc,
                            scalar1=self_sb[:, s, 1:2],
                            scalar2=None, op0=AOT.mult)
                        nc.tensor.matmul(
                            out=z[:], lhsT=ident16[:],
                            rhs=rwS[:].rearrange("p b h -> p (b h)"),
                            start=False, stop=True)
                        out_cb(s, sl, z, og)
                    flush_cb(g0, gs, og)

            # ---------------- phase D: layer 1 ----------------
            h1_view = h1_shard.ap().rearrange("(t p) h -> p t h", p=P)
            with tc.tile_pool(name="d_g", bufs=3) as gpool, \
                 tc.tile_pool(name="d_z", bufs=3, space="PSUM") as zpool, \
                 tc.tile_pool(name="d_s", bufs=4) as spool, \
                 tc.tile_pool(name="d_r", bufs=4) as rpool, \
                 tc.tile_pool(name="d_o", bufs=2) as opool:

                def l1_cb(s, sl, z, og):
                    for br in range(2):
                        dst = og[:, sl, br, :]
                        nc.vector.tensor_tensor(out=dst,
                                                in0=z[:, br * H:(br + 1) * H],
                                                in1=b1sb[:], op=AOT.add)
                        nc.vector.tensor_scalar_max(dst, dst, 0.0)

                def l1_flush(g0, gs, og):
                    nc.sync.dma_start(
                        out=h1_view[:, g0:g0 + gs, :],
                        in_=og[:].rearrange("p t b h -> p t (b h)"))

                msg_pass(hw_full, hw_view, False, l1_cb, l1_flush, gpool,
                         zpool, spool, rpool, opool, "h1g", [2, H], dt16)

            nc.gpsimd.collective_compute(
                "AllGather", AOT.bypass, replica_groups=groups_all,
                ins=[h1_shard.ap().opt()], outs=[h1_full.ap().opt()])

            # ---------------- phase F: layer 2 + combine ----------------
            out_view = outp.ap().rearrange("(t p) o -> p t o", p=P)
            with tc.tile_pool(name="f_g", bufs=3) as gpool, \
                 tc.tile_pool(name="f_z", bufs=3, space="PSUM") as zpool, \
                 tc.tile_pool(name="f_s", bufs=4) as spool, \
                 tc.tile_pool(name="f_r", bufs=4) as rpool, \
                 tc.tile_pool(name="f_o", bufs=2) as opool, \
                 tc.tile_pool(name="f_e", bufs=3) as epool, \
                 tc.tile_pool(name="f_tp", bufs=2, space="PSUM") as tpool, \
                 tc.tile_pool(name="f_op", bufs=2, space="PSUM") as opsp:

                def l2_cb(s, sl, z, og):
                    scr = epool.tile([P, H], dt32, tag="scr")
                    scr2 = epool.tile([P, H], dt32, tag="scr2")
                    lg = epool.tile([P, 1], dt32, tag="lg")
                    lp = epool.tile([P, 1], dt32, tag="lp")
                    nc.vector.tensor_tensor(out=scr[:], in0=z[:, 0:H],
                                            in1=w2dsb[:], op=AOT.mult)
                    nc.vector.tensor_reduce(out=lg[:], in_=scr[:],
                                            axis=mybir.AxisListType.X,
                                            op=AOT.add)
                    nc.vector.tensor_tensor(out=scr2[:], in0=z[:, H:2 * H],
                                            in1=w2dsb[:], op=AOT.mult)
                    nc.vector.tensor_reduce(out=lp[:], in_=scr2[:],
                                            axis=mybir.AxisListType.X,
                                            op=AOT.add)
                    dl = epool.tile([P, 1], dt32, tag="dl")
                    nc.vector.tensor_tensor(out=dl[:], in0=lg[:], in1=lp[:],
                                            op=AOT.subtract)
                    wg = epool.tile([P, 1], dt32, tag="wg")
                    nc.scalar.activation(
                        out=wg[:], in_=dl[:],
                        func=mybir.ActivationFunctionType.Sigmoid)
                    zp = epool.tile([P, H], dt32, tag="zp")
                    nc.vector.tensor_copy(out=zp[:], in_=z[:, H:2 * H])
                    tdiff = epool.tile([P, H], dt32, tag="tdiff")
                    nc.vector.tensor_tensor(out=tdiff[:], in0=z[:, 0:H],
                                            in1=zp[:], op=AOT.subtract)
                    blend = epool.tile([P, H], dt32, tag="blend")
                    nc.vector.scalar_tensor_tensor(
                        out=blend[:], in0=tdiff[:], scalar=wg[:],
                        in1=zp[:], op0=AOT.mult, op1=AOT.add)
                    bT_ps = tpool.tile([P, P], dt32, tag="bT")
                    nc.tensor.transpose(out=bT_ps[:], in_=blend[:],
                                        identity=ident[:])
                    bT = epool.tile([P, P], dt16, tag="bTs")
                    nc.vector.tensor_copy(out=bT[:], in_=bT_ps[:])
                    o_ps = opsp.tile([P, O], dt32, tag="ops")
                    nc.tensor.matmul(out=o_ps[:], lhsT=bT[:], rhs=w2sb[:],
                                     start=True, stop=True)
                    nc.vector.tensor_tensor(out=og[:, sl, :], in0=o_ps[:],
                                            in1=b2sb[:], op=AOT.add)

                def l2_flush(g0, gs, og):
                    nc.sync.dma_start(out=out_view[:, g0:g0 + gs, :],
                                      in_=og[:])

                msg_pass(h1_full, h1_view, True, l2_cb, l2_flush, gpool,
                         zpool, spool, rpool, opool, "outg", [O], dt32)

    nc.compile()
    return nc


_BUILD_CACHE = {}


def _get_program(cfg, K):
    key = (cfg.N, cfg.E, cfg.GG, K)
    if key not in _BUILD_CACHE:
        _BUILD_CACHE[key] = build_program(cfg, K)
    return _BUILD_CACHE[key]


LAST_RESULTS = None


def _run(cfg, inputs):
    from concourse.bass_utils import run_bass_kernel_spmd
    global LAST_RESULTS
    in_maps, K = _preprocess(cfg, **inputs)
    nc = _get_program(cfg, K)
    trace = bool(int(os.environ.get("KERNEL_TRACE", "0")))
    res = run_bass_kernel_spmd(nc, in_maps, core_ids=list(range(cfg.ncores)),
                               trace=trace)
    LAST_RESULTS = res
    out = np.concatenate([res.results[c]["out"] for c in range(cfg.ncores)],
                         axis=0)[:cfg.N]
    return np.ascontiguousarray(out.astype(np.float32))


def kernel(x, edge_index, ppmi_edge_weight, W1, b1, W2, b2, dense_w, dense_b):
    return _run(FULL, dict(x=x, edge_index=edge_index,
                           ppmi_edge_weight=ppmi_edge_weight, W1=W1, b1=b1,
                           W2=W2, b2=b2, dense_w=dense_w, dense_b=dense_b))

